# revision 71
# baseline (speedup 1.0000x reference)
"""Local2d (unshared-weight conv) Bass kernel for 8 trn2 NeuronCores.

Problem: input (64,64,32,32), weight (32,32,128,64,3,3), bias (128,32,32)
-> out (64,128,32,32).  K=3, stride 1, pad 1.

Sharding: spatial over h_out — core i handles output rows 4i..4i+3 and
reads the disjoint weight slice for those rows, plus a 6-row input halo
slab.

The kernel is DMA-bound on the weight stream, so precision is chosen to
minimize bytes within the 2e-2 tolerance:
  - weights: fp8 e3m4, pre-scaled by 32 on host (keeps the randn/24
    values out of the subnormal range); 9.4MB/core.
  - input: fp8 e3m4, pre-scaled by 2; 1.0MB/core incl. an h-shifted
    upper-partition copy of 2 slab rows for the ki-paired matmuls.
  - output: bf16 on device, upcast to f32 on host; 2.1MB/core.
The 64x product scale is NOT descaled on device: the device computes
64*(conv+bias) in bf16 (bias is pre-scaled by 64 on host) and the host
divides by 64 during the f32 upcast — bf16 is floating so the relative
precision is unaffected.  Measured end-to-end rel err ~1.3e-2 vs 2e-2.

Per output location (ho,wo) the contraction (c,ki,kj)=576 accumulates
in PSUM as either 9 K=64 matmuls (rows 0-1) or, for rows 2-3, 3 K=128
ki-paired matmuls + 3 K=64 (6 total; the moving operand's upper 64
partitions hold the slab shifted down one row).  The mix balances the PE
stream against the serialized DMA stream, which is the binding resource
in the cost model: total ~12.6MB/core at ~360 GB/s.
"""

import numpy as np
import ml_dtypes

B, C, O, KK, H, W = 64, 64, 128, 3, 32, 32
HO = WO = 32
NCORES = 8
RPC = HO // NCORES          # output rows per core
LOCS = RPC * WO             # locations per core
G = 8                       # locations per weight-DMA group
NG = LOCS // G


def _build_bass():
    from concourse import bacc
    import concourse.mybir as mybir
    from concourse.tile import TileContext

    f32 = mybir.dt.float32
    bf16 = mybir.dt.bfloat16
    f8 = mybir.dt.float8e3
    nc = bacc.Bacc("TRN2", target_bir_lowering=False, debug=False,
                   num_devices=NCORES)

    # input slab without the zero pad columns (those are memset on-chip):
    # [64(c), 6, 32, 64(b)] e3m4, values pre-scaled by 2.
    slab_d = nc.dram_tensor("slab", (64, RPC + 2, W, B), f8,
                            kind="ExternalInput").ap()
    # weights pre-scaled by 32, e3m4, partition-major per group:
    # [g][c(64 part)][j, ki, kj, o] fully contiguous per-partition runs.
    NGU = 3 * NG // 4           # unpaired groups (rows 0-2)
    wt_d = nc.dram_tensor("wt", (NGU, 64, G, KK, KK, O), f8,
                          kind="ExternalInput").ap()
    # row 3 uses ki-paired K=128 matmuls: weights split into the paired
    # part [(ki0,ki1)xC = 128 partitions, kj, O] and the ki=2 remainder
    wp_d = nc.dram_tensor("wp", (NG - NGU, 128, G, KK, O), f8,
                          kind="ExternalInput").ap()
    ws_d = nc.dram_tensor("ws", (NG - NGU, 64, G, KK, O), f8,
                          kind="ExternalInput").ap()
    bias_d = nc.dram_tensor("bias", (O, LOCS), bf16,
                            kind="ExternalInput").ap()
    # 64x64 identity: lets the PE build the h-shifted upper slab copy
    # on-chip (out partition = stationary free index), so those bytes
    # never cross the serialized DMA device
    id_d = nc.dram_tensor("idm", (64, 64), f8, kind="ExternalInput").ap()
    out_d = nc.dram_tensor("out", (RPC, O, WO, B), bf16,
                           kind="ExternalOutput").ap()

    with TileContext(nc) as tc:
        with tc.tile_pool(name="xslab", bufs=1) as xpool, \
             tc.tile_pool(name="wpool", bufs=8) as wpool, \
             tc.tile_pool(name="bpool", bufs=1) as bpool, \
             tc.tile_pool(name="opool", bufs=4) as opool, \
             tc.tile_pool(name="psum", bufs=8, space="PSUM") as pspool:

            # lower 64 partitions: slab rows 0-5; upper: row 4 at slot 3
            # (the h-shifted copy the ki-paired matmuls read)
            X = xpool.tile([128, RPC + 2, W + 2, B], f8)
            # pad columns 0 and 33 are zeros, built on-chip
            nc.vector.memset(X[:, :, 0:1, :], 0.0)
            nc.vector.memset(X[:, :, W + 1:W + 2, :], 0.0)
            # Every transfer rides the single in-order sync queue, in
            # exactly the order that keeps the PE fed: tiny first weight
            # chunk + just-enough slab, then weight groups with slab rows /
            # bias trickled between them just ahead of first use.
            bias_t = bpool.tile([128, LOCS], bf16)
            idt = bpool.tile([64, 64], f8, name="idt")

            out_rows = {}
            for g in range(NG):
                if g >= NGU:
                    wp = wpool.tile([128, G, KK, O], f8, tag="wp")
                    ws = wpool.tile([64, G, KK, O], f8, tag="ws")
                    nc.sync.dma_start(wp, wp_d[g - NGU])
                    nc.sync.dma_start(ws, ws_d[g - NGU])

                elif g == 0:
                    wt = wpool.tile([64, G, KK, KK, O], f8, tag="wt")
                    # small first chunk so matmuls can start sooner; the
                    # rest of g0 rides the Pool queue whose descriptor-gen
                    # pipelines in parallel with the sync queue's
                    nc.sync.dma_start(wt[:, 0:2], wt_d[g, :, 0:2])
                    nc.sync.dma_start(X[0:64, 0:3, 1:9], slab_d[:, 0:3, 0:8])
                    nc.gpsimd.dma_start(wt[:, 2:G], wt_d[g, :, 2:G])
                    nc.gpsimd.dma_start(bias_t, bias_d)
                    nc.sync.dma_start(X[0:64, 0:3, 9:21], slab_d[:, 0:3, 8:20])
                elif g >= 1:
                    wt = wpool.tile([64, G, KK, KK, O], f8, tag="wt")
                    nc.sync.dma_start(wt, wt_d[g])
                if g == 1:
                    nc.sync.dma_start(X[0:64, 0:3, 21:33], slab_d[:, 0:3, 20:32])
                elif g in (3, 6, 10):
                    # stage slab rows just ahead of first use:
                    # row 3 from loc 32 (hol=1), row 4 from 64, row 5 from 96
                    r = 3 + (g > 3) + (g > 6)
                    nc.sync.dma_start(X[0:64, r:r + 1, 1:33],
                                      slab_d[:, r:r + 1])
                    if g == 3:
                        nc.sync.dma_start(idt, id_d)
                elif g == 7:
                    # build the h-shifted upper copy (slot 3 <- slab row 4)
                    # on the PE+ACT engines instead of DMA: identity matmul
                    # hops the data to partitions 64-127 via PSUM
                    for q in range(8):
                        pst = pspool.tile([128, 4, B], f32, tag="ps4",
                                          name=f"tp{q}")
                        nc.tensor.matmul(pst[64:128],
                                         idt,
                                         X[0:64, 4, 1 + 4 * q:5 + 4 * q, :])
                        nc.scalar.copy(X[64:128, 3, 1 + 4 * q:5 + 4 * q, :],
                                       pst[64:128])

                for j in range(G):
                    loc = g * G + j
                    hol, wo = divmod(loc, WO)
                    if wo == 0:
                        out_rows[hol] = opool.tile([128, WO, B], bf16,
                                                   tag="orow",
                                                   name=f"orow{hol}")
                    orow = out_rows[hol]

                    # pair width 2 for the final locs shortens the tail
                    pw = 4 if loc < LOCS - 8 else 2
                    if wo % pw == 0:
                        ps4 = pspool.tile([128, pw, B], f32, tag="ps4",
                                          name=f"ps{loc}")
                    half = ps4[:, wo % pw, :]
                    if g >= NGU:
                        # ki 0+1 paired across 128 partitions, then ki=2
                        for kj in range(KK):
                            nc.tensor.matmul(half, wp[:, j, kj, :],
                                             X[:, hol, wo + kj, :],
                                             start=(kj == 0), stop=False)
                        for kj in range(KK):
                            nc.tensor.matmul(half, ws[:, j, kj, :],
                                             X[0:64, hol + 2, wo + kj, :],
                                             start=False, stop=(kj == 2))
                    else:
                        n = 0
                        for ki in range(KK):
                            for kj in range(KK):
                                nc.tensor.matmul(half,
                                                 wt[:, j, ki, kj, :],
                                                 X[0:64, hol + ki,
                                                   wo + kj, :],
                                                 start=(n == 0),
                                                 stop=(n == 8))
                                n += 1
                    if wo % pw == pw - 1:
                        nc.vector.tensor_tensor(
                            orow[:, wo - pw + 1:wo + 1, :], ps4,
                            bias_t[:, loc - pw + 1:loc + 1, None]
                            .to_broadcast((128, pw, B)),
                            mybir.AluOpType.add)

            # ALL output flushes are deferred to the sync queue after the
            # last weight DMA: with every weight group resident in SBUF
            # (wpool bufs=16) the weight stream runs gapless, and flushing
            # after it cannot delay any matmul.  The last row goes out in
            # fine-grained chunks so the tail after the final matmul is one
            # small transfer.
            for hol in range(RPC - 1):
                nc.sync.dma_start(out_d[hol], out_rows[hol])
            last = out_rows[RPC - 1]
            for lo, hi in ((0, 8), (8, 16), (16, 24), (24, 28), (28, 32)):
                nc.sync.dma_start(out_d[RPC - 1, :, lo:hi, :],
                                  last[:, lo:hi, :])
    nc.finalize()
    return nc


def _prep_inputs(input, weight, bias):
    inp = np.ascontiguousarray(input, dtype=np.float32)
    bis = np.ascontiguousarray(bias, dtype=np.float32)

    # [h, w, c, b]; input x2 and weight x32 keep e3m4 quantization out of
    # the subnormal range; the 64x product scale is descaled on-device.
    in2 = np.ascontiguousarray((inp * 2.0).transpose(2, 3, 1, 0)).astype(
        ml_dtypes.float8_e3m4)
    w8 = (np.asarray(weight, dtype=np.float32) * 32.0).astype(
        ml_dtypes.float8_e3m4)

    in_maps = []
    for core in range(NCORES):
        h0 = core * RPC
        img = np.zeros((64, RPC + 2, W, B), ml_dtypes.float8_e3m4)
        for hp in range(RPC + 2):
            h = h0 - 1 + hp
            if 0 <= h < H:
                img[:, hp, :, :] = in2[h].transpose(1, 0, 2)
        # rows 0-1: [loc, O, C, ki, kj] -> [g][c][j, ki, kj, o]
        wc = w8[h0:h0 + RPC].reshape(LOCS, O, C, KK, KK)
        NGU = 3 * NG // 4
        wcA, wcB = wc[0:NGU * G], wc[NGU * G:]
        wt = np.ascontiguousarray(
            wcA.transpose(2, 0, 3, 4, 1)         # [c, loc, ki, kj, o]
               .reshape(C, NGU, G, KK, KK, O)
               .transpose(1, 0, 2, 3, 4, 5))     # [g, c, j, ki, kj, o]
        # rows 2-3: ki0/ki1 paired on the partition axis, ki=2 separate
        pk = np.concatenate([wcB[:, :, :, 0, :], wcB[:, :, :, 1, :]],
                            axis=2)              # [loc, O, 128(c,ki), kj]
        wp = np.ascontiguousarray(
            pk.transpose(2, 0, 3, 1)             # [128, loc, kj, O]
              .reshape(128, NG - NGU, G, KK, O)
              .transpose(1, 0, 2, 3, 4))
        ws = np.ascontiguousarray(
            wcB[:, :, :, 2, :].transpose(2, 0, 3, 1)
               .reshape(C, NG - NGU, G, KK, O)
               .transpose(1, 0, 2, 3, 4))
        in_maps.append({
            "slab": img,
            "wt": wt,
            "wp": wp,
            "ws": ws,
            "bias": np.ascontiguousarray(
                bis.reshape(O, HO, WO)[:, h0:h0 + RPC, :] * 64.0)
                .reshape(O, LOCS).astype(ml_dtypes.bfloat16),
        })
    return in_maps


_RUN_KW = {}  # test.py can inject trace=True etc.
_LAST_RESULT = [None]
_NC_CACHE = [None]


def kernel(input, weight, bias):
    from concourse.bass_utils import run_bass_kernel_spmd

    in_maps = _prep_inputs(input, weight, bias)
    if _NC_CACHE[0] is None:
        _NC_CACHE[0] = _build_bass()
    nc = _NC_CACHE[0]
    res = run_bass_kernel_spmd(nc, in_maps, core_ids=list(range(NCORES)),
                               **_RUN_KW)
    _LAST_RESULT[0] = res
    arr = np.stack([np.asarray(r["out"], dtype=np.float32)
                    for r in res.results]) / 64.0     # [core,hol,o,wo,b]
    out = arr.transpose(4, 2, 0, 1, 3).reshape(B, O, HO, WO)
    return np.ascontiguousarray(out)
